# revision 14
# baseline (speedup 1.0000x reference)
"""Chamfer distance kernel for 8 TRN2 NeuronCores (Bass/Tile).

Problem: points1 [16384, 3], points2 [16384, 3] fp32.
  out = sum_i min_j ||p1_i - p2_j|| + sum_j min_i ||p1_i - p2_j||   (scalar)

Strategy
--------
sqrt is monotonic, so min_j ||.|| = sqrt(min_j ||.||^2): only squared
distances are ever materialized, and sqrt runs on the host over the 32K
row-minima.

Squared distances come from a single K=5 matmul with augmented points:
    a_i = [x, y, z, |p|^2, 1]          (lhsT form, stationary)
    b_j = [-2x, -2y, -2z, 1, |p|^2]    (rhs form, moving)
    a_i . b_j = |p1_i|^2 + |p2_j|^2 - 2 p1_i.p2_j = d2(i, j)

Both chamfer terms are row-min problems (term2 is a row-min of the
transposed matrix = distance matrix with roles swapped), so each core
computes row-minima for 2048 rows of D(p1_chunk, p2) and 2048 rows of
D(p2_chunk, p1): 32 row-tiles of 128 rows. Row-min over the free axis is a
native DVE tensor_reduce straight out of PSUM - no partition-axis
reductions, no collectives.

Windowed (KNN) mode: points are sorted by x on the host; each 128-row tile
only scans a contiguous window of W sorted candidates. Exactness is
verified per row on the host (found_min <= margin^2 where margin is the
x-distance to the nearest excluded candidate); rows that fail are
recomputed exactly on the host (rare).
"""

import numpy as np

import concourse.bass as bass
import concourse.mybir as mybir
import concourse.tile as tile
from concourse import bacc
from concourse.bass_utils import run_bass_kernel_spmd

F32 = mybir.dt.float32

N = 16384  # points1 rows
M = 16384  # points2 rows
NCORES = 8
P = 128  # partitions / rows per tile
TILES_PER_DIR = N // NCORES // P  # 16
NT = 2 * TILES_PER_DIR  # 32 row-tiles per core (16 per direction)

# Candidate window per row tile. W == M -> brute force over all candidates.
WINDOW = 2048
CHUNK = 2048  # psum chunk: 4 banks of 512 fp32


def _build_nc(window: int, chunk: int = CHUNK, repeats: int = 1):
    """Build the SPMD program (same for all cores).

    Inputs (per core):
      lhs_aug [NT, 5, 128]     augmented row tiles (lhsT form)
      rhs_win [NT, 5, window]  augmented candidate windows (rhs form)
    Output:
      minima  [128, NT]        min d2 per row of each tile
    """
    assert window % 512 == 0 and chunk % 512 == 0
    n_chunks = (window + chunk - 1) // chunk

    brute = window >= M

    nc = bacc.Bacc(
        "TRN2",
        target_bir_lowering=False,
        debug=False,
        enable_asserts=True,
        num_devices=NCORES,
    )
    lhs_ap = nc.dram_tensor("lhs_aug", [NT, 5, P], F32, kind="ExternalInput").ap()
    rhs_shape = [2, 5, M] if brute else [NT, 5, window]
    rhs_ap = nc.dram_tensor("rhs_win", rhs_shape, F32, kind="ExternalInput").ap()
    out_ap = nc.dram_tensor("minima", [P, NT], F32, kind="ExternalOutput").ap()

    with tile.TileContext(nc) as tc:
        with (
            tc.tile_pool(name="lhs", bufs=4) as lhs_pool,
            tc.tile_pool(name="rhs", bufs=1 if brute else 3) as rhs_pool,
            tc.tile_pool(name="psum", bufs=2, space="PSUM") as psum_pool,
            tc.tile_pool(name="stage", bufs=3) as stage_pool,
            tc.tile_pool(name="outp", bufs=1) as out_pool,
        ):
            if brute:
                rhs_full = []
                for d in range(2):
                    rf = rhs_pool.tile([5, M], F32, tag=f"rhs{d}")
                    nc.sync.dma_start(rf[:], rhs_ap[d])
                    rhs_full.append(rf)
            out_sb = out_pool.tile([P, NT], F32)
            for t in _rep_tiles(repeats):
                lhs_sb = lhs_pool.tile([5, P], F32)
                nc.sync.dma_start(lhs_sb[:], lhs_ap[t])
                if brute:
                    rhs_sb = rhs_full[0] if t < TILES_PER_DIR else rhs_full[1]
                else:
                    rhs_sb = rhs_pool.tile([5, window], F32)
                    nc.sync.dma_start(rhs_sb[:], rhs_ap[t])

                if n_chunks > 1:
                    stage = stage_pool.tile([P, n_chunks], F32)
                for cb in range(n_chunks):
                    cw = min(chunk, window - cb * chunk)
                    ps = psum_pool.tile([P, chunk], F32, tag="ps")
                    for k in range(0, cw, 512):
                        nc.tensor.matmul(
                            ps[:, k : k + 512],
                            lhs_sb[:],
                            rhs_sb[:, cb * chunk + k : cb * chunk + k + 512],
                            start=True,
                            stop=True,
                        )
                    dst = stage[:, cb : cb + 1] if n_chunks > 1 else out_sb[:, t : t + 1]
                    nc.vector.tensor_reduce(
                        dst,
                        ps[:, :cw],
                        axis=mybir.AxisListType.X,
                        op=mybir.AluOpType.min,
                    )
                if n_chunks > 1:
                    nc.vector.tensor_reduce(
                        out_sb[:, t : t + 1],
                        stage[:],
                        axis=mybir.AxisListType.X,
                        op=mybir.AluOpType.min,
                    )
            nc.sync.dma_start(out_ap[:], out_sb[:])

    nc.compile()
    return nc


def _rep_tiles(repeats):
    for _ in range(repeats):
        yield from range(NT)


def _augment(pts):
    """pts [n, 3] f32 -> (A [5, n] lhsT form, B [5, n] rhs form, sq [n])."""
    x = np.ascontiguousarray(pts, dtype=np.float32)
    sq = (x[:, 0] * x[:, 0] + x[:, 1] * x[:, 1] + x[:, 2] * x[:, 2]).astype(
        np.float32
    )
    ones = np.ones_like(sq)
    A = np.stack([x[:, 0], x[:, 1], x[:, 2], sq, ones], axis=0)
    B = np.stack([-2.0 * x[:, 0], -2.0 * x[:, 1], -2.0 * x[:, 2], ones, sq], axis=0)
    return np.ascontiguousarray(A), np.ascontiguousarray(B), sq


_NC_CACHE: dict = {}


def _get_nc(window: int, repeats: int = 1):
    key = (window, repeats)
    nc = _NC_CACHE.get(key)
    if nc is None:
        nc = _build_nc(window, repeats=repeats)
        _NC_CACHE[key] = nc
    return nc


def _prepare_inputs(points1, points2, window: int):
    """Host-side shard/window prep. Returns (in_maps, meta) where meta holds
    what's needed to finish/verify on the host."""
    p1 = np.ascontiguousarray(points1, dtype=np.float32)
    p2 = np.ascontiguousarray(points2, dtype=np.float32)

    if window >= M:
        ord1 = np.arange(N)
        ord2 = np.arange(M)
    else:
        ord1 = np.argsort(p1[:, 0], kind="stable")
        ord2 = np.argsort(p2[:, 0], kind="stable")
    s1 = p1[ord1]
    s2 = p2[ord2]
    A1, B1, _ = _augment(s1)
    A2, B2, _ = _augment(s2)

    n_tiles_total = N // P  # 128 row tiles per direction
    # Window start (in sorted candidate ranks) per global row tile.
    if window >= M:
        c0_1 = np.zeros(n_tiles_total, dtype=np.int64)  # p1 tiles scan all p2
        c0_2 = np.zeros(n_tiles_total, dtype=np.int64)
    else:

        def _starts(xs_rows, xs_cands):
            starts = np.empty(n_tiles_total, dtype=np.int64)
            for g in range(n_tiles_total):
                lo = np.searchsorted(xs_cands, xs_rows[g * P])
                hi = np.searchsorted(xs_cands, xs_rows[(g + 1) * P - 1])
                c = (lo + hi) // 2 - window // 2
                starts[g] = min(max(c, 0), len(xs_cands) - window)
            return starts

        c0_1 = _starts(s1[:, 0], s2[:, 0])
        c0_2 = _starts(s2[:, 0], s1[:, 0])

    brute = window >= M
    rhs_brute = np.stack([B2, B1]) if brute else None
    in_maps = []
    for c in range(NCORES):
        lhs = np.empty((NT, 5, P), dtype=np.float32)
        rhs = rhs_brute if brute else np.empty((NT, 5, window), dtype=np.float32)
        for tl in range(TILES_PER_DIR):
            g = c * TILES_PER_DIR + tl
            lhs[tl] = A1[:, g * P : (g + 1) * P]
            lhs[TILES_PER_DIR + tl] = A2[:, g * P : (g + 1) * P]
            if not brute:
                rhs[tl] = B2[:, c0_1[g] : c0_1[g] + window]
                rhs[TILES_PER_DIR + tl] = B1[:, c0_2[g] : c0_2[g] + window]
        in_maps.append({"lhs_aug": lhs, "rhs_win": rhs})

    meta = dict(s1=s1, s2=s2, c0_1=c0_1, c0_2=c0_2, window=window)
    return in_maps, meta


def _finish(results, meta):
    """Gather per-core minima, verify window margins, fall back exactly where
    needed, and return the chamfer sum."""
    window = meta["window"]
    s1, s2 = meta["s1"], meta["s2"]
    n_tiles_total = N // P

    # m1[g*P + p] = min d2 for sorted-p1 row of global tile g, partition p.
    m1 = np.empty(N, dtype=np.float32)
    m2 = np.empty(M, dtype=np.float32)
    for c in range(NCORES):
        mins = results[c]["minima"]  # [P, NT]
        for tl in range(TILES_PER_DIR):
            g = c * TILES_PER_DIR + tl
            m1[g * P : (g + 1) * P] = mins[:, tl]
            m2[g * P : (g + 1) * P] = mins[:, TILES_PER_DIR + tl]

    if window < M:

        def _verify_fix(mvals, rows, cands, c0s):
            xs_r = rows[:, 0]
            xs_c = cands[:, 0]
            ncand = len(xs_c)
            starts = np.repeat(c0s, P)
            left = np.where(
                starts > 0, xs_r - xs_c[np.maximum(starts - 1, 0)], np.inf
            )
            ends = starts + window
            right = np.where(
                ends < ncand, xs_c[np.minimum(ends, ncand - 1)] - xs_r, np.inf
            )
            margin = np.minimum(left, right)
            bad = ~(mvals <= (margin * margin))
            nbad = int(bad.sum())
            if nbad:
                d = (
                    rows[bad, None, :].astype(np.float64)
                    - cands[None, :, :].astype(np.float64)
                ) ** 2
                mvals[bad] = d.sum(-1).min(1).astype(np.float32)
            return nbad

        nb1 = _verify_fix(m1, s1, s2, meta["c0_1"])
        nb2 = _verify_fix(m2, s2, s1, meta["c0_2"])
        _finish.fallback_rows = nb1 + nb2
    else:
        _finish.fallback_rows = 0

    total = np.sqrt(np.maximum(m1, 0.0).astype(np.float64)).sum() + np.sqrt(
        np.maximum(m2, 0.0).astype(np.float64)
    ).sum()
    return np.float32(total)


_EXEC_CACHE: dict = {}


def _get_exec(window: int, repeats: int = 1):
    """Build (once) a persistent jitted shard_map executable for the program.

    Mirrors concourse.bass2jax.run_bass_via_pjrt, but caches the jitted
    callable so repeat calls don't re-trace. `repeats` selects a program
    variant with the whole tile loop unrolled `repeats` times (for timing:
    slope between repeats=R and repeats=1 isolates pure kernel time).
    """
    key = (window, repeats)
    if key in _EXEC_CACHE:
        return _EXEC_CACHE[key]

    import jax
    from jax.sharding import Mesh, PartitionSpec
    from jax.experimental.shard_map import shard_map

    from concourse.bass2jax import (
        _bass_exec_p,
        install_neuronx_cc_hook,
        partition_id_tensor,
    )

    nc = _get_nc(window, repeats)
    install_neuronx_cc_hook()
    assert nc.dbg_addr is None
    partition_name = (
        nc.partition_id_tensor.name if nc.partition_id_tensor is not None else None
    )

    in_names, out_names, out_avals, zero_shapes = [], [], [], []
    for alloc in nc.m.functions[0].allocations:
        if not isinstance(alloc, mybir.MemoryLocationSet):
            continue
        name = alloc.memorylocations[0].name
        if alloc.kind == "ExternalInput":
            if name != partition_name:
                in_names.append(name)
        elif alloc.kind == "ExternalOutput":
            shape = tuple(alloc.tensor_shape)
            dtype = mybir.dt.np(alloc.dtype)
            out_names.append(name)
            out_avals.append(jax.core.ShapedArray(shape, dtype))
            zero_shapes.append((shape, dtype))
    n_params = len(in_names)
    all_names = in_names + out_names
    if partition_name is not None:
        all_names = all_names + [partition_name]
    all_names = tuple(all_names)

    def _body(*args):
        operands = list(args)
        if partition_name is not None:
            operands.append(partition_id_tensor())
        outs = _bass_exec_p.bind(
            *operands,
            out_avals=tuple(out_avals),
            in_names=all_names,
            out_names=tuple(out_names),
            lowering_input_output_aliases=(),
            sim_require_finite=True,
            sim_require_nnan=True,
            nc=nc,
        )
        return tuple(outs)

    devices = jax.devices()[:NCORES]
    mesh = Mesh(np.asarray(devices), ("core",))
    n_outs = len(out_names)
    donate = tuple(range(n_params, n_params + n_outs))

    fn = jax.jit(
        shard_map(
            _body,
            mesh=mesh,
            in_specs=(PartitionSpec("core"),) * (n_params + n_outs),
            out_specs=(PartitionSpec("core"),) * n_outs,
            check_rep=False,
        ),
        donate_argnums=donate,
        keep_unused=True,
    )

    info = dict(
        nc=nc,
        mesh=mesh,
        in_names=in_names,
        out_names=out_names,
        out_avals=out_avals,
        zero_shapes=zero_shapes,
        n_params=n_params,
        fn=fn,
    )
    _EXEC_CACHE[key] = info
    return info


def _concat_inputs(info, in_maps):
    return [
        np.concatenate([np.asarray(m[name]) for m in in_maps], axis=0)
        for name in info["in_names"]
    ]


def _zeros(info):
    return [
        np.zeros((NCORES * s[0], *s[1:]), d) for (s, d) in info["zero_shapes"]
    ]


def _execute(info, concat_in):
    import jax

    out_arrs = jax.block_until_ready(info["fn"](*concat_in, *_zeros(info)))
    return out_arrs


def _split_results(info, out_arrs):
    results = []
    for c in range(NCORES):
        results.append(
            {
                name: np.asarray(out_arrs[i]).reshape(
                    NCORES, *info["out_avals"][i].shape
                )[c]
                for i, name in enumerate(info["out_names"])
            }
        )
    return results


def _run(points1, points2, window=WINDOW, trace=False):
    info = _get_exec(window)
    in_maps, meta = _prepare_inputs(points1, points2, window)
    out_arrs = _execute(info, _concat_inputs(info, in_maps))
    results = _split_results(info, out_arrs)
    out = _finish(results, meta)
    return out, results


def _time_exec(info, concat_in, reps):
    import time

    best = float("inf")
    for _ in range(reps):
        t0 = time.perf_counter()
        _execute(info, concat_in)
        best = min(best, time.perf_counter() - t0)
    return best


def _bench(points1, points2, window=WINDOW, repeats=5, reps=5):
    """Per-execution kernel time (ns) via the repeated-program slope:
    T = (wall(program x R) - wall(program x 1)) / (R - 1)."""
    import jax
    from jax.sharding import NamedSharding, PartitionSpec

    in_maps, _ = _prepare_inputs(points1, points2, window)

    walls = {}
    for r in (1, repeats):
        info = _get_exec(window, repeats=r)
        sharding = NamedSharding(info["mesh"], PartitionSpec("core"))
        concat_in = [
            jax.device_put(x, sharding) for x in _concat_inputs(info, in_maps)
        ]
        _execute(info, concat_in)  # warm
        walls[r] = _time_exec(info, concat_in, reps)

    per_exec_ns = (walls[repeats] - walls[1]) / (repeats - 1) * 1e9
    return per_exec_ns, walls[1] * 1e9


def kernel(points1, points2):
    out, _ = _run(points1, points2)
    return out
